# revision 22
# baseline (speedup 1.0000x reference)
"""Two-layer mean-aggregation GNN on 8 Trainium2 NeuronCores.

Strategy (node partition, per the sharding hint):
  - Nodes 1D-partitioned: core c owns nodes [c*6250, (c+1)*6250).
  - Edges partitioned by dst owner, sorted by dst tile (128 dst nodes
    per tile).  Per tile, edges are split A/B by src LOCAL row
    (src % 6250 < 3072), so gather indices into the two allgathered
    tables (8*3072 and 8*3200 rows) fit in int16.
  - segment_sum runs on the TensorEngine as one-hot fp8 matmuls
    (DoubleRow fp8 perf mode pairs adjacent 128-edge blocks).  The
    one-hot masks and the per-edge gathered x rows (fp8) are prebuilt
    on the host and streamed.
  - Layer 1 computes h TRANSPOSED ([hid, node] in SBUF) via
    W1-as-stationary matmuls, so no h roundtrip/transpose is needed;
    hW = h @ W2_bot (fp8) is produced per tile and AllGathered in two
    halves: AG_A fires mid-L1 (after tile 23), AG_B after L1.
  - Layer 2 runs in two passes: pass A (src rows in table A) has its
    dma_gathers streaming under the tail of L1 (emission interleaved),
    partial sums staged in SBUF bf16; pass B finishes after AG_B.
  - dma_gathers round-robin over 4 SWDGE queues.
"""

import os
import sys

for _p in ("/opt/trn_rl_repo", "/root/.axon_site/_ro/trn_rl_repo"):
    if os.path.isdir(_p) and _p not in sys.path:
        sys.path.append(_p)

import numpy as np

import concourse.bacc as bacc
import concourse.mybir as mybir
import concourse.tile as tile
import concourse.bass_utils as bass_utils

F32 = mybir.dt.float32
BF16 = mybir.dt.bfloat16
FP8 = mybir.dt.float8e4
I16 = mybir.dt.int16
NP_BF16 = mybir.dt.np(BF16)
NP_FP8 = mybir.dt.np(FP8)
ONE_FP8 = int(np.array(1.0, NP_FP8).view(np.uint8))

AluOp = mybir.AluOpType
ActFn = mybir.ActivationFunctionType
DR = mybir.MatmulPerfMode.DoubleRow

NCORES = 8
N = 50000
E = 800000
FIN = 128
FHID = 256
FOUT = 256
NPC = N // NCORES            # 6250 nodes per core
T = (NPC + 127) // 128       # 49 dst tiles per core
NPAD = T * 128               # 6272
A_T = 24                     # tiles 0..23 are the "A" half
A_ROWS = A_T * 128           # 3072 local rows in table A
B_ROWS = NPAD - A_ROWS       # 3200 local rows in table B
TBL_A = NCORES * A_ROWS      # 24576 (< 32768: int16 gather indices)
TBL_B = NCORES * B_ROWS      # 25600
B_MAX = 56                   # max 128-edge blocks per chunk
MAX_G_BLK = 8                # max blocks per dma_gather call (ring cap)
N_SWDGE_Q = 4                # SWDGE queues; gathers round-robin


def _plan(src, dst):
    """Partition edges by dst owner; per dst tile split by src A/B half.

    Returns (layout, meta, per_core) where layout keys the build cache,
    meta drives program construction, per_core holds edge data.
    """
    core_of = dst // NPC
    per_core = []
    na_ct = np.zeros((NCORES, T), np.int64)
    nb_ct = np.zeros((NCORES, T), np.int64)
    for c in range(NCORES):
        m = core_of == c
        es = src[m].astype(np.int64)
        ed = (dst[m] - c * NPC).astype(np.int64)
        order = np.argsort(ed, kind="stable")
        es, ed = es[order], ed[order]
        tl = ed >> 7
        bounds = np.searchsorted(tl, np.arange(T + 1))
        aa, bb = [], []
        for t in range(T):
            a, b = int(bounds[t]), int(bounds[t + 1])
            sl_es, sl_ed = es[a:b], ed[a:b]
            am = (sl_es % NPC) < A_ROWS
            aa.append((sl_es[am], sl_ed[am]))
            bb.append((sl_es[~am], sl_ed[~am]))
            na_ct[c, t] = int(am.sum())
            nb_ct[c, t] = int((~am).sum())
        per_core.append((aa, bb))

    cap_a = np.maximum(1, -(-na_ct.max(axis=0) // 128))   # blocks, >= 1
    cap_b = -(-nb_ct.max(axis=0) // 128)

    chunks, cur, cur_blk = [], [], 0
    for t in range(T):
        tb = int(cap_a[t] + cap_b[t])
        if cur and cur_blk + tb > B_MAX:
            chunks.append(cur)
            cur, cur_blk = [], 0
        cur.append(t)
        cur_blk += tb
    if cur:
        chunks.append(cur)

    meta = []
    pos = 0
    for tlist in chunks:
        na = int(sum(cap_a[t] for t in tlist))
        nb = int(sum(cap_b[t] for t in tlist))
        tiles = []
        a0, b0 = 0, 0
        for t in tlist:
            tiles.append((t, a0, int(cap_a[t]), b0, int(cap_b[t])))
            a0 += int(cap_a[t])
            b0 += int(cap_b[t])
        meta.append(dict(pos0=pos, na=na, nb=nb, nblk=na + nb, tiles=tiles))
        pos += (na + nb) * 128
    layout = (tuple(int(v) for v in cap_a),
              tuple(int(v) for v in cap_b),
              tuple(tuple(tl) for tl in chunks))
    return layout, meta, per_core, pos


def _fill_core(meta, aa, bb, npos):
    gsrc = np.zeros(npos, np.int64)   # global src id per position (0 on pads)
    idx2 = np.zeros(npos, np.int64)   # table-local gather index (layer 2)
    dloc = np.full(npos, -1, np.int64)
    for ch in meta:
        for (t, a0, anb, b0, bnb) in ch["tiles"]:
            es, ed = aa[t]
            k = len(es)
            if k:
                base = ch["pos0"] + a0 * 128
                gsrc[base:base + k] = es
                idx2[base:base + k] = (es // NPC) * A_ROWS + (es % NPC)
                dloc[base:base + k] = ed - t * 128
            es2, ed2 = bb[t]
            k2 = len(es2)
            if k2:
                base2 = ch["pos0"] + (ch["na"] + b0) * 128
                gsrc[base2:base2 + k2] = es2
                idx2[base2:base2 + k2] = \
                    (es2 // NPC) * B_ROWS + (es2 % NPC) - A_ROWS
                dloc[base2:base2 + k2] = ed2 - t * 128

    assert idx2.max() < 32768

    def wrap(seq):
        w = seq.astype(np.int16).reshape(-1, 16).T  # [16, npos/16]
        return np.ascontiguousarray(np.tile(w, (8, 1)))

    jj = np.nonzero(dloc >= 0)[0]
    m_u8 = np.zeros((128, npos), np.uint8)
    m_u8[jj % 128, (jj // 128) * 128 + dloc[jj]] = ONE_FP8
    return wrap(idx2), m_u8.view(NP_FP8), gsrc


def _build(layout):
    cap_a, cap_b, chunks = layout
    nblk_tot = int(sum(cap_a) + sum(cap_b))
    npos = nblk_tot * 128

    # Rebuild chunk meta (same as _plan).
    meta = []
    pos = 0
    for tlist in chunks:
        na = int(sum(cap_a[t] for t in tlist))
        nb = int(sum(cap_b[t] for t in tlist))
        tiles = []
        a0, b0 = 0, 0
        for t in tlist:
            tiles.append((t, a0, int(cap_a[t]), b0, int(cap_b[t])))
            a0 += int(cap_a[t])
            b0 += int(cap_b[t])
        meta.append(dict(pos0=pos, na=na, nb=nb, nblk=na + nb, tiles=tiles))
        pos += (na + nb) * 128
    assert pos == npos
    nchunks = len(meta)

    nc = bacc.Bacc("TRN2", target_bir_lowering=False, debug=False,
                   enable_asserts=False, num_devices=NCORES,
                   num_swdge_queues=N_SWDGE_Q)

    xe_d = nc.dram_tensor("xe", [128, nblk_tot, FIN], FP8,
                          kind="ExternalInput").ap()
    xT_d = nc.dram_tensor("xT", [128, NPAD], BF16, kind="ExternalInput").ap()
    w1t_d = nc.dram_tensor("w1t", [128, FHID], BF16, kind="ExternalInput").ap()
    w1b_d = nc.dram_tensor("w1b", [128, FHID], BF16, kind="ExternalInput").ap()
    w2t_d = nc.dram_tensor("w2t", [128, 2 * FOUT], BF16, kind="ExternalInput").ap()
    w2b_d = nc.dram_tensor("w2b", [128, 2 * FOUT], BF16, kind="ExternalInput").ap()
    b1_d = nc.dram_tensor("b1", [1, FHID], BF16, kind="ExternalInput").ap()
    b2_d = nc.dram_tensor("b2", [1, FOUT], BF16, kind="ExternalInput").ap()
    invb_d = nc.dram_tensor("invb", [128, NPAD], F32, kind="ExternalInput").ap()
    invp_d = nc.dram_tensor("invp", [128, T], F32, kind="ExternalInput").ap()
    i2_d = nc.dram_tensor("i2", [128, npos // 16], I16, kind="ExternalInput").ap()
    m_d = nc.dram_tensor("mpk", [128, npos], FP8, kind="ExternalInput").ap()
    out_d = nc.dram_tensor("out", [NPAD, FOUT], F32, kind="ExternalOutput").ap()

    def ts(t):
        return slice(t * 128, (t + 1) * 128)

    def pair_blocks(*ranges):
        out = []
        for base, n in ranges:
            b = base
            while b + 1 < base + n:
                out.append((b, 2))
                b += 2
            if b < base + n:
                out.append((b, 1))
        return out

    def m3(mt, b, k):
        return mt[:, b * 128:(b + k) * 128].rearrange(
            "p (b k) -> p b k", k=128)

    gq = [0]

    def emit_gathers(g, src_view, idx_tile, pos0, nblk, out_blk0, elem):
        ncalls = -(-nblk // MAX_G_BLK)
        done = 0
        for i in range(ncalls):
            nb = (nblk - done + (ncalls - i) - 1) // (ncalls - i)
            nidx = nb * 128
            s0 = (pos0 + done * 128) // 16
            nc.gpsimd.dma_gather(
                g[:, out_blk0 + done:out_blk0 + done + nb, :], src_view,
                idx_tile[:, s0:s0 + nidx // 16], nidx, nidx, elem,
                queue_num=gq[0])
            gq[0] = (gq[0] + 1) % N_SWDGE_Q
            done += nb

    with tile.TileContext(nc) as tc:
        with tc.tile_pool(name="const", bufs=1) as cpool, \
             tc.tile_pool(name="dram", bufs=1, space="DRAM") as dpool, \
             tc.tile_pool(name="g1", bufs=2) as g1pool, \
             tc.tile_pool(name="m1", bufs=2) as m1pool, \
             tc.tile_pool(name="g2", bufs=3) as g2pool, \
             tc.tile_pool(name="m2", bufs=3) as m2pool, \
             tc.tile_pool(name="hn", bufs=2) as hnpool, \
             tc.tile_pool(name="hw", bufs=3) as hwpool, \
             tc.tile_pool(name="ot", bufs=3) as otpool, \
             tc.tile_pool(name="pagg", bufs=3, space="PSUM") as pagg_pool, \
             tc.tile_pool(name="pself", bufs=2, space="PSUM") as pself_pool, \
             tc.tile_pool(name="ph", bufs=2, space="PSUM") as ph_pool:
            xT = cpool.tile([128, NPAD], BF16)
            nc.sync.dma_start(xT[:], xT_d)
            w1t = cpool.tile([128, FHID], BF16)
            nc.sync.dma_start(w1t[:], w1t_d)
            w1b = cpool.tile([128, FHID], BF16)
            nc.sync.dma_start(w1b[:], w1b_d)
            w2t = cpool.tile([128, 2 * FOUT], BF16)
            nc.sync.dma_start(w2t[:], w2t_d)
            w2b = cpool.tile([128, 2 * FOUT], BF16)
            nc.sync.dma_start(w2b[:], w2b_d)
            b1s = cpool.tile([1, FHID], BF16)
            nc.sync.dma_start(b1s[:], b1_d)
            b2s = cpool.tile([1, FOUT], BF16)
            nc.sync.dma_start(b2s[:], b2_d)
            invb = cpool.tile([128, NPAD], F32)
            nc.sync.dma_start(invb[:], invb_d)
            invp = cpool.tile([128, T], F32)
            nc.sync.dma_start(invp[:], invp_d)
            i2 = cpool.tile([128, npos // 16], I16)
            nc.sync.dma_start(i2[:], i2_d)
            ones = cpool.tile([1, 128], BF16)
            nc.vector.memset(ones[:], 1.0)
            hTa = cpool.tile([128, NPAD], BF16)
            hTb = cpool.tile([128, NPAD], BF16)
            aggsb = cpool.tile([128, T * FOUT], BF16)

            hwbA = dpool.tile([A_ROWS, FOUT], FP8)
            hwbB = dpool.tile([B_ROWS, FOUT], FP8)
            hwfA = dpool.tile([TBL_A, FOUT], FP8, addr_space="Shared")
            hwfB = dpool.tile([TBL_B, FOUT], FP8, addr_space="Shared")

            # ---------------- Layer 1 (emits AG_A after tile 23) ---------
            def l1_chunk(ch):
                g = g1pool.tile([128, ch["nblk"], FIN], FP8, tag="g1")
                mt = m1pool.tile([128, ch["nblk"] * 128], FP8, tag="m1")
                nc.sync.dma_start(
                    mt[:], m_d[:, ch["pos0"]:ch["pos0"] + ch["nblk"] * 128])
                blk0 = ch["pos0"] // 128
                nc.sync.dma_start(g[:], xe_d[:, blk0:blk0 + ch["nblk"], :])
                for (t, a0, anb, b0, bnb) in ch["tiles"]:
                    paggT = pagg_pool.tile([128, 128], F32, tag="pagg")
                    groups = pair_blocks((a0, anb), (ch["na"] + b0, bnb))
                    for i, (b, k) in enumerate(groups):
                        nc.tensor.matmul(
                            paggT[:],
                            g[:, b, :] if k == 1 else g[:, b:b + 2, :],
                            m3(mt, b, 1)[:, 0, :] if k == 1 else m3(mt, b, 2),
                            start=(i == 0), stop=(i == len(groups) - 1),
                            perf_mode=None if k == 1 else DR)
                    hn = hnpool.tile([128, 128], BF16, tag="hn")
                    nc.vector.tensor_tensor(
                        hn[:], paggT[:], invb[:, ts(t)], AluOp.mult)
                    for h, hT in ((0, hTa), (1, hTb)):
                        hsl = slice(h * 128, (h + 1) * 128)
                        psT = pself_pool.tile([128, 128], F32, tag="pself")
                        nc.tensor.matmul(psT[:], b1s[:1, hsl], ones[:1, :],
                                         start=True, stop=False)
                        nc.tensor.matmul(psT[:], w1t[:, hsl], xT[:, ts(t)],
                                         start=False, stop=False)
                        nc.tensor.matmul(psT[:], w1b[:, hsl], hn[:],
                                         start=False, stop=True)
                        nc.scalar.activation(hT[:, ts(t)], psT[:], ActFn.Relu)
                    ph = ph_pool.tile([128, FOUT], F32, tag="ph")
                    nc.tensor.matmul(ph[:], hTa[:, ts(t)], w2b[:, 0:FOUT],
                                     start=True, stop=False)
                    nc.tensor.matmul(ph[:], hTb[:, ts(t)], w2b[:, FOUT:],
                                     start=False, stop=True)
                    hw = hwpool.tile([128, FOUT], FP8, tag="hw")
                    nc.vector.tensor_copy(hw[:], ph[:])
                    if t < A_T:
                        nc.sync.dma_start(hwbA[ts(t), :], hw[:])
                    else:
                        tb = t - A_T
                        nc.sync.dma_start(hwbB[ts(tb), :], hw[:])
                    if t == A_T - 1:
                        nc.gpsimd.collective_compute(
                            "AllGather", AluOp.bypass,
                            replica_groups=[list(range(NCORES))],
                            ins=[hwbA.opt()], outs=[hwfA.opt()])

            # ---------------- Layer 2 pass A (gathers from table A) ------
            def l2a_chunk(ch):
                if ch["na"] == 0:
                    return
                g = g2pool.tile([128, ch["na"], FOUT], FP8, tag="g2")
                mt = m2pool.tile([128, ch["na"] * 128], FP8, tag="m2")
                nc.sync.dma_start(
                    mt[:], m_d[:, ch["pos0"]:ch["pos0"] + ch["na"] * 128])
                emit_gathers(g, hwfA, i2, ch["pos0"], ch["na"], 0, FOUT)
                for (t, a0, anb, b0, bnb) in ch["tiles"]:
                    pagg = pagg_pool.tile([128, FOUT], F32, tag="pagg")
                    groups = pair_blocks((a0, anb))
                    for i, (b, k) in enumerate(groups):
                        nc.tensor.matmul(
                            pagg[:],
                            m3(mt, b, 1)[:, 0, :] if k == 1 else m3(mt, b, 2),
                            g[:, b, :] if k == 1 else g[:, b:b + 2, :],
                            start=(i == 0), stop=(i == len(groups) - 1),
                            perf_mode=None if k == 1 else DR)
                    nc.vector.tensor_copy(
                        aggsb[:, t * FOUT:(t + 1) * FOUT], pagg[:])

            # ---------------- Layer 2 pass B + combine -------------------
            def l2b_chunk(ch):
                gb = None
                if ch["nb"]:
                    gb = g2pool.tile([128, ch["nb"], FOUT], FP8, tag="g2")
                    mt = m2pool.tile([128, ch["nb"] * 128], FP8, tag="m2")
                    posb = ch["pos0"] + ch["na"] * 128
                    nc.sync.dma_start(
                        mt[:], m_d[:, posb:posb + ch["nb"] * 128])
                    emit_gathers(gb, hwfB, i2, posb, ch["nb"], 0, FOUT)
                for (t, a0, anb, b0, bnb) in ch["tiles"]:
                    ps2 = pself_pool.tile([128, FOUT], F32, tag="pself")
                    nc.tensor.matmul(ps2[:], ones[:1, :], b2s[:1, :],
                                     start=True, stop=False)
                    nc.tensor.matmul(ps2[:], hTa[:, ts(t)], w2t[:, 0:FOUT],
                                     start=False, stop=False)
                    nc.tensor.matmul(ps2[:], hTb[:, ts(t)], w2t[:, FOUT:],
                                     start=False, stop=True)
                    agg_sl = aggsb[:, t * FOUT:(t + 1) * FOUT]
                    t1 = otpool.tile([128, FOUT], F32, tag="t1")
                    if bnb:
                        pagg = pagg_pool.tile([128, FOUT], F32, tag="pagg")
                        groups = pair_blocks((b0, bnb))
                        for i, (b, k) in enumerate(groups):
                            nc.tensor.matmul(
                                pagg[:],
                                m3(mt, b, 1)[:, 0, :] if k == 1
                                else m3(mt, b, 2),
                                gb[:, b, :] if k == 1 else gb[:, b:b + 2, :],
                                start=(i == 0), stop=(i == len(groups) - 1),
                                perf_mode=None if k == 1 else DR)
                        s1 = otpool.tile([128, FOUT], F32, tag="s1")
                        nc.vector.tensor_tensor(
                            s1[:], pagg[:], agg_sl, AluOp.add)
                        nc.vector.tensor_scalar(
                            t1[:], s1[:], invp[:, t:t + 1], None, AluOp.mult)
                    else:
                        nc.vector.tensor_scalar(
                            t1[:], agg_sl, invp[:, t:t + 1], None, AluOp.mult)
                    o1 = otpool.tile([128, FOUT], F32, tag="o1")
                    nc.vector.tensor_tensor(o1[:], t1[:], ps2[:], AluOp.add)
                    o2 = otpool.tile([128, FOUT], F32, tag="o2")
                    nc.scalar.activation(o2[:], o1[:], ActFn.Relu)
                    nc.sync.dma_start(out_d[ts(t), :], o2[:])

            # Emission order: L1 chunks until tile 23 is emitted (AG_A is
            # emitted inside l1_chunk), then interleave pass-A chunks with
            # the remaining L1 chunks so pass-A gathers stream under L1.
            k1 = next(i for i, ch in enumerate(meta)
                      if any(t == A_T - 1 for (t, *_ ) in ch["tiles"])) + 1
            for ch in meta[:k1]:
                l1_chunk(ch)
            rest = meta[k1:]
            ia = 0
            for ch in rest:
                l1_chunk(ch)
                if ia < nchunks:
                    l2a_chunk(meta[ia])
                    ia += 1
            while ia < nchunks:
                l2a_chunk(meta[ia])
                ia += 1

            nc.gpsimd.collective_compute(
                "AllGather", AluOp.bypass,
                replica_groups=[list(range(NCORES))],
                ins=[hwbB.opt()], outs=[hwfB.opt()])

            for ch in meta:
                l2b_chunk(ch)

    nc.compile()
    return nc


_CACHE = {}


def _run(inputs, trace=False):
    x = np.asarray(inputs["x"], np.float32)
    src = np.asarray(inputs["src"])
    dst = np.asarray(inputs["dst"])
    W1 = np.asarray(inputs["W1"], np.float32)
    b1 = np.asarray(inputs["b1"], np.float32)
    W2 = np.asarray(inputs["W2"], np.float32)
    b2 = np.asarray(inputs["b2"], np.float32)

    deg = np.bincount(dst, minlength=N).astype(np.float64)
    inv_deg = np.where(deg > 0, 1.0 / np.maximum(deg, 1.0), 0.0).astype(np.float32)

    layout, meta, per_core, npos = _plan(src, dst)
    if layout not in _CACHE:
        _CACHE[layout] = _build(layout)
    nc = _CACHE[layout]

    x_bf = x.astype(NP_BF16)
    x_f8 = x.astype(NP_FP8)
    w1t = np.ascontiguousarray(W1[0:128]).astype(NP_BF16)
    w1b = np.ascontiguousarray(W1[128:256]).astype(NP_BF16)
    w2t = np.ascontiguousarray(
        np.concatenate([W2[0:128], W2[128:256]], axis=1)).astype(NP_BF16)
    w2b = np.ascontiguousarray(
        np.concatenate([W2[256:384], W2[384:512]], axis=1)).astype(NP_BF16)
    b1r = b1.reshape(1, FHID).astype(NP_BF16)
    b2r = b2.reshape(1, FOUT).astype(NP_BF16)

    in_maps = []
    for c in range(NCORES):
        aa, bb = per_core[c]
        i2w, mpk, gsrc = _fill_core(meta, aa, bb, npos)
        xe = np.ascontiguousarray(
            x_f8[gsrc].reshape(npos // 128, 128, FIN).transpose(1, 0, 2))
        xTc = np.zeros((128, NPAD), NP_BF16)
        xTc[:, :NPC] = x_bf[c * NPC:(c + 1) * NPC].T
        iv = np.zeros(NPAD, np.float32)
        iv[:NPC] = inv_deg[c * NPC:(c + 1) * NPC]
        invb = np.ascontiguousarray(np.tile(iv, (128, 1)))
        invp = np.ascontiguousarray(iv.reshape(T, 128).T)
        in_maps.append({
            "xe": xe, "xT": xTc,
            "w1t": w1t, "w1b": w1b, "w2t": w2t, "w2b": w2b,
            "b1": b1r, "b2": b2r,
            "invb": invb, "invp": invp,
            "i2": i2w, "mpk": mpk,
        })

    res = bass_utils.run_bass_kernel_spmd(
        nc, in_maps, core_ids=list(range(NCORES)), trace=trace)
    out = np.concatenate(
        [res.results[c]["out"][:NPC] for c in range(NCORES)], axis=0)
    return np.ascontiguousarray(out.astype(np.float32)), res


def kernel(**inputs):
    out, _ = _run(inputs, trace=False)
    return out


# revision 49
# speedup vs baseline: 1.4027x; 1.4027x over previous
"""Two-layer mean-aggregation GNN on 8 Trainium2 NeuronCores.

Strategy (matches the node-partition sharding hint):
  - Nodes are 1D-partitioned: core c owns nodes [c*6250, (c+1)*6250).
  - Edges are partitioned by dst owner and sorted by dst tile (128 dst
    nodes per tile). Per tile, edges are split into "low"/"high" source
    ranges so gather indices fit in int16, and padded to 128-multiples.
  - segment_sum is computed on the TensorEngine as one-hot selection
    matmuls: gathered src rows [128 edges, d] x one-hot M [128 edges,
    128 dst] accumulate into PSUM.  M (the graph structure, fp8 0/1) is
    prebuilt on the host and streamed.
  - Layer 1 aggregates raw x (gathered via dma_gather from a replicated
    bf16 copy), applies inv_deg, and feeds both concat halves through
    W1 as two PSUM-accumulated matmuls.  h stays on-device.
  - Between layers each core computes hW = h @ W2_bot for its own nodes
    and AllGathers hW (bf16) so layer 2 can aggregate pre-transformed
    rows directly (no transpose needed in the layer-2 inner loop).
  - Weights are tiny and replicated to every core.
"""

import os
import sys

for _p in ("/opt/trn_rl_repo", "/root/.axon_site/_ro/trn_rl_repo"):
    if os.path.isdir(_p) and _p not in sys.path:
        sys.path.append(_p)

import numpy as np

import concourse.bacc as bacc
import concourse.mybir as mybir
import concourse.tile as tile
import concourse.bass_utils as bass_utils

F32 = mybir.dt.float32
BF16 = mybir.dt.bfloat16
FP8 = mybir.dt.float8e4
I16 = mybir.dt.int16
NP_BF16 = mybir.dt.np(BF16)
NP_FP8 = mybir.dt.np(FP8)
ONE_FP8 = int(np.array(1.0, NP_FP8).view(np.uint8))

AluOp = mybir.AluOpType
ActFn = mybir.ActivationFunctionType

NCORES = 8
N = 50000
E = 800000
FIN = 128
FHID = 256
FOUT = 256
NPC = N // NCORES            # 6250 nodes per core
T = (NPC + 127) // 128       # 49 dst tiles per core
NPAD = T * 128               # 6272
HWROWS = NCORES * NPAD       # 50176 rows in allgathered hW
SRC_BOUND = 32640            # low/high src split (fits int16 in both spaces)
HW_BOUND = (SRC_BOUND // NPC) * NPAD + (SRC_BOUND % NPC)  # 32750
B_MAX = 56                   # max 128-edge blocks per gather chunk
MAX_G_BLK = 8               # max blocks per dma_gather call (SWDGE ring cap)
N_SWDGE_Q = 4               # SWDGE queues; gathers round-robin across them
DMA_SCRATCH = 16384          # SWDGE ring carveout (descs = /16 per queue)
RES_CH = 10                  # leading chunks whose masks stay SBUF-resident


def _hwrow(s):
    s = s.astype(np.int64)
    return (s // NPC) * NPAD + (s % NPC)


def _plan(src, dst):
    """Partition/sort edges; derive the shared (cross-core) static layout.

    Returns (layout, per_core) where layout drives program construction and
    per_core holds the edge data for input-tensor fill.
    """
    core_of = dst // NPC
    per_core = []
    nlow_ct = np.zeros((NCORES, T), np.int64)
    nhigh_ct = np.zeros((NCORES, T), np.int64)
    for c in range(NCORES):
        m = core_of == c
        es = src[m].astype(np.int64)
        ed = (dst[m] - c * NPC).astype(np.int64)
        order = np.argsort(ed, kind="stable")
        es, ed = es[order], ed[order]
        tl = ed >> 7
        bounds = np.searchsorted(tl, np.arange(T + 1))
        lows, highs = [], []
        for t in range(T):
            a, b = int(bounds[t]), int(bounds[t + 1])
            sl_es, sl_ed = es[a:b], ed[a:b]
            lm = sl_es < SRC_BOUND
            lows.append((sl_es[lm], sl_ed[lm]))
            highs.append((sl_es[~lm], sl_ed[~lm]))
            nlow_ct[c, t] = int(lm.sum())
            nhigh_ct[c, t] = int((~lm).sum())
        per_core.append((lows, highs))

    cap_low = np.maximum(1, -(-nlow_ct.max(axis=0) // 128))   # blocks
    cap_high = -(-nhigh_ct.max(axis=0) // 128)

    # Greedy chunking of consecutive tiles.
    chunks, cur, cur_blk = [], [], 0
    for t in range(T):
        tb = int(cap_low[t] + cap_high[t])
        if cur and cur_blk + tb > B_MAX:
            chunks.append(cur)
            cur, cur_blk = [], 0
        cur.append(t)
        cur_blk += tb
    if cur:
        chunks.append(cur)

    meta = []
    pos = 0
    for tlist in chunks:
        nlow = int(sum(cap_low[t] for t in tlist))
        nhigh = int(sum(cap_high[t] for t in tlist))
        tiles = []
        lo, hi = 0, nlow
        for t in tlist:
            tiles.append((t, lo, int(cap_low[t]), hi, int(cap_high[t])))
            lo += int(cap_low[t])
            hi += int(cap_high[t])
        meta.append(dict(pos0=pos, nlow=nlow, nhigh=nhigh,
                         nblk=nlow + nhigh, tiles=tiles))
        pos += (nlow + nhigh) * 128
    layout = (tuple(int(v) for v in cap_low),
              tuple(int(v) for v in cap_high),
              tuple(tuple(tl) for tl in chunks))
    return layout, meta, per_core, pos


def _fill_core(meta, lows, highs, npos):
    gsrc = np.zeros(npos, np.int64)   # global src id per position (0 on pads)
    idx2 = np.zeros(npos, np.int64)   # hw-space local gather index (layer 2)
    dloc = np.full(npos, -1, np.int64)
    for ch in meta:
        for (t, lo, lnb, hi, hnb) in ch["tiles"]:
            es, ed = lows[t]
            k = len(es)
            if k:
                base = ch["pos0"] + lo * 128
                gsrc[base:base + k] = es
                idx2[base:base + k] = _hwrow(es)
                dloc[base:base + k] = ed - t * 128
            es2, ed2 = highs[t]
            k2 = len(es2)
            if k2:
                base2 = ch["pos0"] + hi * 128
                gsrc[base2:base2 + k2] = es2
                idx2[base2:base2 + k2] = _hwrow(es2) - HW_BOUND
                dloc[base2:base2 + k2] = ed2 - t * 128

    assert idx2.max() < 32768

    def wrap(seq):
        w = seq.astype(np.int16).reshape(-1, 16).T  # [16, npos/16]
        return np.ascontiguousarray(np.tile(w, (8, 1)))

    jj = np.nonzero(dloc >= 0)[0]
    m_u8 = np.zeros((128, npos), np.uint8)
    m_u8[jj % 128, (jj // 128) * 128 + dloc[jj]] = ONE_FP8
    return wrap(idx2), m_u8.view(NP_FP8), gsrc


def _build(layout):
    cap_low, cap_high, chunks = layout
    nblk_tot = int(sum(cap_low) + sum(cap_high))
    npos = nblk_tot * 128

    # Rebuild chunk meta (same as _plan).
    meta = []
    pos = 0
    for tlist in chunks:
        nlow = int(sum(cap_low[t] for t in tlist))
        nhigh = int(sum(cap_high[t] for t in tlist))
        tiles = []
        lo, hi = 0, nlow
        for t in tlist:
            tiles.append((t, lo, int(cap_low[t]), hi, int(cap_high[t])))
            lo += int(cap_low[t])
            hi += int(cap_high[t])
        meta.append(dict(pos0=pos, nlow=nlow, nhigh=nhigh,
                         nblk=nlow + nhigh, tiles=tiles))
        pos += (nlow + nhigh) * 128
    assert pos == npos

    nc = bacc.Bacc("TRN2", target_bir_lowering=False, debug=False,
                   enable_asserts=False, num_devices=NCORES,
                   num_swdge_queues=N_SWDGE_Q,
                   dynamic_dma_scratch_size=DMA_SCRATCH)

    xe_d = nc.dram_tensor("xe", [128, nblk_tot, FIN], FP8,
                          kind="ExternalInput").ap()
    xT_d = nc.dram_tensor("xT", [128, NPAD], BF16, kind="ExternalInput").ap()
    w1t_d = nc.dram_tensor("w1t", [128, FHID], BF16, kind="ExternalInput").ap()
    w1b_d = nc.dram_tensor("w1b", [128, FHID], BF16, kind="ExternalInput").ap()
    w2t_d = nc.dram_tensor("w2t", [128, 2 * FOUT], BF16, kind="ExternalInput").ap()
    w2b_d = nc.dram_tensor("w2b", [128, 2 * FOUT], BF16, kind="ExternalInput").ap()
    b1c_d = nc.dram_tensor("b1c", [128, 2], F32, kind="ExternalInput").ap()
    b2_d = nc.dram_tensor("b2", [1, FOUT], BF16, kind="ExternalInput").ap()
    invb_d = nc.dram_tensor("invb", [128, NPAD], BF16, kind="ExternalInput").ap()
    invp_d = nc.dram_tensor("invp", [128, T], F32, kind="ExternalInput").ap()
    i2_d = nc.dram_tensor("i2", [128, npos // 16], I16, kind="ExternalInput").ap()
    m_d = nc.dram_tensor("mpk", [128, npos], FP8, kind="ExternalInput").ap()
    out_d = nc.dram_tensor("out", [NPAD, FOUT], BF16, kind="ExternalOutput").ap()

    def ts(t):
        return slice(t * 128, (t + 1) * 128)

    def pair_blocks(lo, lnb, hi, hnb):
        # Pair consecutive blocks within each src range for DoubleRow.
        out = []
        for base, n in ((lo, lnb), (hi, hnb)):
            b = base
            while b + 1 < base + n:
                out.append((b, 2))
                b += 2
            if b < base + n:
                out.append((b, 1))
        return out

    DR = mybir.MatmulPerfMode.DoubleRow

    def m3(mt, b, k):
        # [128, k, 128] view of mask blocks b..b+k
        return mt[:, b * 128:(b + k) * 128].rearrange(
            "p (b k) -> p b k", k=128)

    gq = [0]  # round-robin SWDGE queue cursor

    def emit_gathers(g, src_view, idx_tile, idx_base, pos0, nblk, out_blk0,
                     elem):
        # Split a gather region into ring-capacity-sized dma_gather calls,
        # evenly (per-call cost is mostly fixed, avoid tiny tail calls).
        ncalls = -(-nblk // MAX_G_BLK)
        done = 0
        for i in range(ncalls):
            nb = (nblk - done + (ncalls - i) - 1) // (ncalls - i)
            nidx = nb * 128
            s0 = (pos0 + done * 128 - idx_base) // 16
            nc.gpsimd.dma_gather(
                g[:, out_blk0 + done:out_blk0 + done + nb, :], src_view,
                idx_tile[:, s0:s0 + nidx // 16], nidx, nidx, elem,
                queue_num=gq[0])
            gq[0] = (gq[0] + 1) % N_SWDGE_Q
            done += nb

    with tile.TileContext(nc) as tc:
        with tc.tile_pool(name="const", bufs=1) as cpool, \
             tc.tile_pool(name="dram", bufs=1, space="DRAM") as dpool, \
             tc.tile_pool(name="g", bufs=3) as gpool, \
             tc.tile_pool(name="mm", bufs=3) as mpool:
            xT = cpool.tile([128, NPAD], BF16)
            nc.sync.dma_start(xT[:], xT_d)
            w1t = cpool.tile([128, FHID], BF16)
            nc.sync.dma_start(w1t[:], w1t_d)
            w1b = cpool.tile([128, FHID], BF16)
            nc.sync.dma_start(w1b[:], w1b_d)
            w2t = cpool.tile([128, 2 * FOUT], BF16)
            nc.sync.dma_start(w2t[:], w2t_d)
            w2b = cpool.tile([128, 2 * FOUT], BF16)
            nc.sync.dma_start(w2b[:], w2b_d)
            b1c = cpool.tile([128, 2], F32)
            nc.sync.dma_start(b1c[:], b1c_d)
            b2s = cpool.tile([1, FOUT], BF16)
            nc.sync.dma_start(b2s[:], b2_d)
            invb = cpool.tile([128, NPAD], BF16)
            nc.sync.dma_start(invb[:], invb_d)
            invp = cpool.tile([128, T], F32)
            nc.sync.dma_start(invp[:], invp_d)
            i2 = cpool.tile([128, npos // 16], I16)
            nc.sync.dma_start(i2[:], i2_d)
            ones = cpool.tile([1, 128], BF16)
            nc.vector.memset(ones[:], 1.0)

            hTa = cpool.tile([128, NPAD], BF16)
            hTb = cpool.tile([128, NPAD], BF16)
            hwb = dpool.tile([NPAD, FOUT], FP8)
            hwf = dpool.tile([HWROWS, FOUT], FP8, addr_space="Shared")
            mres = []

            # ------------- Layer 1 (h computed transposed) + hW ----------
            with tc.tile_pool(name="paggT", bufs=2, space="PSUM") as paggT_pool, \
                 tc.tile_pool(name="pself", bufs=2, space="PSUM") as pself_pool, \
                 tc.tile_pool(name="phw", bufs=2, space="PSUM") as phw_pool, \
                 tc.tile_pool(name="hn", bufs=2) as hnpool, \
                 tc.tile_pool(name="hwsb", bufs=3) as hwpool:
                for ci, ch in enumerate(meta):
                    g = gpool.tile([128, ch["nblk"], FIN], FP8, tag="g")
                    if ci < RES_CH:
                        mt = cpool.tile([128, ch["nblk"] * 128], FP8,
                                        tag=f"mres{ci}")
                        mres.append(mt)
                    else:
                        mt = mpool.tile([128, ch["nblk"] * 128], FP8, tag="m")
                    nc.sync.dma_start(
                        mt[:], m_d[:, ch["pos0"]:ch["pos0"] + ch["nblk"] * 128])
                    blk0 = ch["pos0"] // 128
                    nc.sync.dma_start(
                        g[:], xe_d[:, blk0:blk0 + ch["nblk"], :])
                    for (t, lo, lnb, hi, hnb) in ch["tiles"]:
                        paggT = paggT_pool.tile([128, 128], F32, tag="paggT")
                        groups = pair_blocks(lo, lnb, hi, hnb)
                        for i, (b, k) in enumerate(groups):
                            nc.tensor.matmul(
                                paggT[:],
                                g[:, b, :] if k == 1 else g[:, b:b + 2, :],
                                m3(mt, b, 1)[:, 0, :] if k == 1 else m3(mt, b, 2),
                                start=(i == 0), stop=(i == len(groups) - 1),
                                perf_mode=None if k == 1 else DR)
                        hn = hnpool.tile([128, 128], BF16, tag="hn")
                        nc.vector.tensor_tensor(
                            hn[:], paggT[:], invb[:, ts(t)], AluOp.mult)
                        for h, hT in ((0, hTa), (1, hTb)):
                            hsl = slice(h * 128, (h + 1) * 128)
                            psT = pself_pool.tile([128, 128], F32, tag="pself")
                            nc.tensor.matmul(psT[:], w1t[:, hsl], xT[:, ts(t)],
                                             start=True, stop=False)
                            nc.tensor.matmul(psT[:], w1b[:, hsl], hn[:],
                                             start=False, stop=True)
                            nc.scalar.activation(hT[:, ts(t)], psT[:],
                                                 ActFn.Relu,
                                                 bias=b1c[:, h:h + 1])
                        ph = phw_pool.tile([128, FOUT], F32, tag="phw")
                        nc.tensor.matmul(ph[:], hTa[:, ts(t)], w2b[:, 0:FOUT],
                                         start=True, stop=False)
                        nc.tensor.matmul(ph[:], hTb[:, ts(t)],
                                         w2b[:, FOUT:2 * FOUT],
                                         start=False, stop=True)
                        hw = hwpool.tile([128, FOUT], FP8, tag="hw")
                        nc.vector.tensor_copy(hw[:], ph[:])
                        nc.sync.dma_start(hwb[ts(t), :], hw[:])

            # Prefetch a few post-resident mask chunks before the collective
            # (DMA serializes around collectives; these land pre-AllGather).
            mpre = {}
            if True:
                for ci in range(RES_CH, min(RES_CH + 3, len(meta))):
                    ch = meta[ci]
                    mt = mpool.tile([128, ch["nblk"] * 128], FP8, tag="m")
                    nc.sync.dma_start(
                        mt[:],
                        m_d[:, ch["pos0"]:ch["pos0"] + ch["nblk"] * 128])
                    mpre[ci] = mt

                nc.gpsimd.collective_compute(
                    "AllGather", AluOp.bypass,
                    replica_groups=[list(range(NCORES))],
                    ins=[hwb.opt()], outs=[hwf.opt()])

                # ---------------- Layer 2 ----------------
                with tc.tile_pool(name="pagg2", bufs=2, space="PSUM") as pagg2_pool, \
                     tc.tile_pool(name="pself2", bufs=2, space="PSUM") as pself2_pool, \
                     tc.tile_pool(name="t1sb", bufs=3) as t1pool, \
                     tc.tile_pool(name="osb", bufs=3) as opool:
                    for ci, ch in enumerate(meta):
                        g = gpool.tile([128, ch["nblk"], FOUT], FP8, tag="g")
                        if ci < RES_CH:
                            mt = mres[ci]
                        elif ci in mpre:
                            mt = mpre.pop(ci)
                        else:
                            mt = mpool.tile([128, ch["nblk"] * 128], FP8,
                                            tag="m")
                            nc.sync.dma_start(
                                mt[:], m_d[:, ch["pos0"]:ch["pos0"] +
                                           ch["nblk"] * 128])
                        if ch["nlow"]:
                            emit_gathers(g, hwf[0:HW_BOUND, :], i2, 0,
                                         ch["pos0"], ch["nlow"], 0, FOUT)
                        if ch["nhigh"]:
                            emit_gathers(g, hwf[HW_BOUND:HWROWS, :], i2, 0,
                                         ch["pos0"] + ch["nlow"] * 128,
                                         ch["nhigh"], ch["nlow"], FOUT)
                        for (t, lo, lnb, hi, hnb) in ch["tiles"]:
                            pagg = pagg2_pool.tile([128, FOUT], F32,
                                                   tag="pagg2")
                            groups = pair_blocks(lo, lnb, hi, hnb)
                            for i, (b, k) in enumerate(groups):
                                nc.tensor.matmul(
                                    pagg[:],
                                    m3(mt, b, 1)[:, 0, :] if k == 1
                                    else m3(mt, b, 2),
                                    g[:, b, :] if k == 1 else g[:, b:b + 2, :],
                                    start=(i == 0),
                                    stop=(i == len(groups) - 1),
                                    perf_mode=None if k == 1 else DR)
                            ps2 = pself2_pool.tile([128, FOUT], F32,
                                                   tag="pself2")
                            nc.tensor.matmul(ps2[:], ones[:1, :], b2s[:1, :],
                                             start=True, stop=False)
                            nc.tensor.matmul(ps2[:], hTa[:, ts(t)],
                                             w2t[:, 0:FOUT],
                                             start=False, stop=False)
                            nc.tensor.matmul(ps2[:], hTb[:, ts(t)],
                                             w2t[:, FOUT:2 * FOUT],
                                             start=False, stop=True)
                            t1 = t1pool.tile([128, FOUT], F32, tag="t1")
                            nc.vector.tensor_scalar(
                                t1[:], pagg[:], invp[:, t:t + 1], None,
                                AluOp.mult)
                            o1 = opool.tile([128, FOUT], F32, tag="o1")
                            nc.vector.tensor_tensor(o1[:], t1[:], ps2[:],
                                                    AluOp.add)
                            o2 = opool.tile([128, FOUT], BF16, tag="o2")
                            nc.scalar.activation(o2[:], o1[:], ActFn.Relu)
                            nc.sync.dma_start(out_d[ts(t), :], o2[:])

    nc.compile()
    return nc


_CACHE = {}


def _run(inputs, trace=False):
    x = np.asarray(inputs["x"], np.float32)
    src = np.asarray(inputs["src"])
    dst = np.asarray(inputs["dst"])
    W1 = np.asarray(inputs["W1"], np.float32)
    b1 = np.asarray(inputs["b1"], np.float32)
    W2 = np.asarray(inputs["W2"], np.float32)
    b2 = np.asarray(inputs["b2"], np.float32)

    deg = np.bincount(dst, minlength=N).astype(np.float64)
    inv_deg = np.where(deg > 0, 1.0 / np.maximum(deg, 1.0), 0.0).astype(np.float32)

    layout, meta, per_core, npos = _plan(src, dst)
    if layout not in _CACHE:
        _CACHE[layout] = _build(layout)
    nc = _CACHE[layout]

    x_bf = x.astype(NP_BF16)
    w1t = np.ascontiguousarray(W1[0:128]).astype(NP_BF16)
    w1b = np.ascontiguousarray(W1[128:256]).astype(NP_BF16)
    w2t = np.ascontiguousarray(
        np.concatenate([W2[0:128], W2[128:256]], axis=1)).astype(NP_BF16)
    w2b = np.ascontiguousarray(
        np.concatenate([W2[256:384], W2[384:512]], axis=1)).astype(NP_BF16)
    b1cr = np.ascontiguousarray(b1.reshape(2, 128).T.astype(np.float32))
    b2r = b2.reshape(1, FOUT).astype(NP_BF16)

    x_f8 = x.astype(NP_FP8)
    in_maps = []
    for c in range(NCORES):
        lows, highs = per_core[c]
        i2w, mpk, gsrc = _fill_core(meta, lows, highs, npos)
        xe = np.ascontiguousarray(
            x_f8[gsrc].reshape(npos // 128, 128, FIN).transpose(1, 0, 2))
        xTc = np.zeros((128, NPAD), NP_BF16)
        xTc[:, :NPC] = x_bf[c * NPC:(c + 1) * NPC].T
        iv = np.zeros(NPAD, np.float32)
        iv[:NPC] = inv_deg[c * NPC:(c + 1) * NPC]
        invb = np.ascontiguousarray(np.tile(iv, (128, 1))).astype(NP_BF16)
        invp = np.ascontiguousarray(iv.reshape(T, 128).T)
        in_maps.append({
            "xe": xe, "xT": xTc,
            "w1t": w1t, "w1b": w1b, "w2t": w2t, "w2b": w2b,
            "b1c": b1cr, "b2": b2r,
            "invb": invb, "invp": invp,
            "i2": i2w, "mpk": mpk,
        })

    res = bass_utils.run_bass_kernel_spmd(
        nc, in_maps, core_ids=list(range(NCORES)), trace=trace)
    out = np.concatenate(
        [res.results[c]["out"][:NPC] for c in range(NCORES)], axis=0)
    return np.ascontiguousarray(out.astype(np.float32)), res


def kernel(**inputs):
    out, _ = _run(inputs, trace=False)
    return out



# revision 50
# speedup vs baseline: 1.4637x; 1.0435x over previous
"""Two-layer mean-aggregation GNN on 8 Trainium2 NeuronCores.

Strategy (matches the node-partition sharding hint):
  - Nodes are 1D-partitioned: core c owns nodes [c*6250, (c+1)*6250).
  - Edges are partitioned by dst owner and sorted by dst tile (128 dst
    nodes per tile). Per tile, edges are split into "low"/"high" source
    ranges so gather indices fit in int16, and padded to 128-multiples.
  - segment_sum is computed on the TensorEngine as one-hot selection
    matmuls: gathered src rows [128 edges, d] x one-hot M [128 edges,
    128 dst] accumulate into PSUM.  M (the graph structure, fp8 0/1) is
    prebuilt on the host and streamed.
  - Layer 1 aggregates raw x (gathered via dma_gather from a replicated
    bf16 copy), applies inv_deg, and feeds both concat halves through
    W1 as two PSUM-accumulated matmuls.  h stays on-device.
  - Between layers each core computes hW = h @ W2_bot for its own nodes
    and AllGathers hW (bf16) so layer 2 can aggregate pre-transformed
    rows directly (no transpose needed in the layer-2 inner loop).
  - Weights are tiny and replicated to every core.
"""

import os
import sys

for _p in ("/opt/trn_rl_repo", "/root/.axon_site/_ro/trn_rl_repo"):
    if os.path.isdir(_p) and _p not in sys.path:
        sys.path.append(_p)

import numpy as np

import concourse.bacc as bacc
import concourse.mybir as mybir
import concourse.tile as tile
import concourse.bass_utils as bass_utils

F32 = mybir.dt.float32
BF16 = mybir.dt.bfloat16
FP8 = mybir.dt.float8e4
I16 = mybir.dt.int16
NP_BF16 = mybir.dt.np(BF16)
NP_FP8 = mybir.dt.np(FP8)
ONE_FP8 = int(np.array(1.0, NP_FP8).view(np.uint8))

AluOp = mybir.AluOpType
ActFn = mybir.ActivationFunctionType

NCORES = 8
N = 50000
E = 800000
FIN = 128
FHID = 256
FOUT = 256
NPC = N // NCORES            # 6250 nodes per core
T = (NPC + 127) // 128       # 49 dst tiles per core
NPAD = T * 128               # 6272
HWROWS = NCORES * NPAD       # 50176 rows in allgathered hW
SRC_BOUND = 32640            # low/high src split (fits int16 in both spaces)
HW_BOUND = (SRC_BOUND // NPC) * NPAD + (SRC_BOUND % NPC)  # 32750
B_MAX = 72                   # max 128-edge blocks per gather chunk
MAX_G_BLK = 8               # max blocks per dma_gather call (SWDGE ring cap)
N_SWDGE_Q = 4               # SWDGE queues; gathers round-robin across them
DMA_SCRATCH = 16384          # SWDGE ring carveout (descs = /16 per queue)
RES_CH = 8                   # leading chunks whose masks stay SBUF-resident


def _hwrow(s):
    s = s.astype(np.int64)
    return (s // NPC) * NPAD + (s % NPC)


def _plan(src, dst):
    """Partition/sort edges; derive the shared (cross-core) static layout.

    Returns (layout, per_core) where layout drives program construction and
    per_core holds the edge data for input-tensor fill.
    """
    core_of = dst // NPC
    per_core = []
    nlow_ct = np.zeros((NCORES, T), np.int64)
    nhigh_ct = np.zeros((NCORES, T), np.int64)
    for c in range(NCORES):
        m = core_of == c
        es = src[m].astype(np.int64)
        ed = (dst[m] - c * NPC).astype(np.int64)
        order = np.argsort(ed, kind="stable")
        es, ed = es[order], ed[order]
        tl = ed >> 7
        bounds = np.searchsorted(tl, np.arange(T + 1))
        lows, highs = [], []
        for t in range(T):
            a, b = int(bounds[t]), int(bounds[t + 1])
            sl_es, sl_ed = es[a:b], ed[a:b]
            lm = sl_es < SRC_BOUND
            lows.append((sl_es[lm], sl_ed[lm]))
            highs.append((sl_es[~lm], sl_ed[~lm]))
            nlow_ct[c, t] = int(lm.sum())
            nhigh_ct[c, t] = int((~lm).sum())
        per_core.append((lows, highs))

    cap_low = np.maximum(1, -(-nlow_ct.max(axis=0) // 128))   # blocks
    cap_high = -(-nhigh_ct.max(axis=0) // 128)

    # Greedy chunking of consecutive tiles.
    chunks, cur, cur_blk = [], [], 0
    for t in range(T):
        tb = int(cap_low[t] + cap_high[t])
        if cur and cur_blk + tb > B_MAX:
            chunks.append(cur)
            cur, cur_blk = [], 0
        cur.append(t)
        cur_blk += tb
    if cur:
        chunks.append(cur)

    meta = []
    pos = 0
    for tlist in chunks:
        nlow = int(sum(cap_low[t] for t in tlist))
        nhigh = int(sum(cap_high[t] for t in tlist))
        tiles = []
        lo, hi = 0, nlow
        for t in tlist:
            tiles.append((t, lo, int(cap_low[t]), hi, int(cap_high[t])))
            lo += int(cap_low[t])
            hi += int(cap_high[t])
        meta.append(dict(pos0=pos, nlow=nlow, nhigh=nhigh,
                         nblk=nlow + nhigh, tiles=tiles))
        pos += (nlow + nhigh) * 128
    layout = (tuple(int(v) for v in cap_low),
              tuple(int(v) for v in cap_high),
              tuple(tuple(tl) for tl in chunks))
    return layout, meta, per_core, pos


def _fill_core(meta, lows, highs, npos):
    gsrc = np.zeros(npos, np.int64)   # global src id per position (0 on pads)
    idx2 = np.zeros(npos, np.int64)   # hw-space local gather index (layer 2)
    dloc = np.full(npos, -1, np.int64)
    for ch in meta:
        for (t, lo, lnb, hi, hnb) in ch["tiles"]:
            es, ed = lows[t]
            k = len(es)
            if k:
                base = ch["pos0"] + lo * 128
                gsrc[base:base + k] = es
                idx2[base:base + k] = _hwrow(es)
                dloc[base:base + k] = ed - t * 128
            es2, ed2 = highs[t]
            k2 = len(es2)
            if k2:
                base2 = ch["pos0"] + hi * 128
                gsrc[base2:base2 + k2] = es2
                idx2[base2:base2 + k2] = _hwrow(es2) - HW_BOUND
                dloc[base2:base2 + k2] = ed2 - t * 128

    assert idx2.max() < 32768

    def wrap(seq):
        w = seq.astype(np.int16).reshape(-1, 16).T  # [16, npos/16]
        return np.ascontiguousarray(np.tile(w, (8, 1)))

    jj = np.nonzero(dloc >= 0)[0]
    m_u8 = np.zeros((128, npos), np.uint8)
    m_u8[jj % 128, (jj // 128) * 128 + dloc[jj]] = ONE_FP8
    return wrap(idx2), m_u8.view(NP_FP8), gsrc


def _build(layout):
    cap_low, cap_high, chunks = layout
    nblk_tot = int(sum(cap_low) + sum(cap_high))
    npos = nblk_tot * 128

    # Rebuild chunk meta (same as _plan).
    meta = []
    pos = 0
    for tlist in chunks:
        nlow = int(sum(cap_low[t] for t in tlist))
        nhigh = int(sum(cap_high[t] for t in tlist))
        tiles = []
        lo, hi = 0, nlow
        for t in tlist:
            tiles.append((t, lo, int(cap_low[t]), hi, int(cap_high[t])))
            lo += int(cap_low[t])
            hi += int(cap_high[t])
        meta.append(dict(pos0=pos, nlow=nlow, nhigh=nhigh,
                         nblk=nlow + nhigh, tiles=tiles))
        pos += (nlow + nhigh) * 128
    assert pos == npos

    nc = bacc.Bacc("TRN2", target_bir_lowering=False, debug=False,
                   enable_asserts=False, num_devices=NCORES,
                   num_swdge_queues=N_SWDGE_Q,
                   dynamic_dma_scratch_size=DMA_SCRATCH)

    xe_d = nc.dram_tensor("xe", [128, nblk_tot, FIN], FP8,
                          kind="ExternalInput").ap()
    xT_d = nc.dram_tensor("xT", [128, NPAD], BF16, kind="ExternalInput").ap()
    w1t_d = nc.dram_tensor("w1t", [128, FHID], BF16, kind="ExternalInput").ap()
    w1b_d = nc.dram_tensor("w1b", [128, FHID], BF16, kind="ExternalInput").ap()
    w2t_d = nc.dram_tensor("w2t", [128, 2 * FOUT], BF16, kind="ExternalInput").ap()
    w2b_d = nc.dram_tensor("w2b", [128, 2 * FOUT], BF16, kind="ExternalInput").ap()
    b1c_d = nc.dram_tensor("b1c", [128, 2], F32, kind="ExternalInput").ap()
    b2_d = nc.dram_tensor("b2", [1, FOUT], BF16, kind="ExternalInput").ap()
    invb_d = nc.dram_tensor("invb", [128, NPAD], BF16, kind="ExternalInput").ap()
    invp_d = nc.dram_tensor("invp", [128, T], F32, kind="ExternalInput").ap()
    i2_d = nc.dram_tensor("i2", [128, npos // 16], I16, kind="ExternalInput").ap()
    m_d = nc.dram_tensor("mpk", [128, npos], FP8, kind="ExternalInput").ap()
    out_d = nc.dram_tensor("out", [NPAD, FOUT], BF16, kind="ExternalOutput").ap()

    def ts(t):
        return slice(t * 128, (t + 1) * 128)

    def pair_blocks(lo, lnb, hi, hnb):
        # Pair consecutive blocks within each src range for DoubleRow.
        out = []
        for base, n in ((lo, lnb), (hi, hnb)):
            b = base
            while b + 1 < base + n:
                out.append((b, 2))
                b += 2
            if b < base + n:
                out.append((b, 1))
        return out

    DR = mybir.MatmulPerfMode.DoubleRow

    def m3(mt, b, k):
        # [128, k, 128] view of mask blocks b..b+k
        return mt[:, b * 128:(b + k) * 128].rearrange(
            "p (b k) -> p b k", k=128)

    gq = [0]  # round-robin SWDGE queue cursor

    def emit_gathers(g, src_view, idx_tile, idx_base, pos0, nblk, out_blk0,
                     elem):
        # Split a gather region into ring-capacity-sized dma_gather calls,
        # evenly (per-call cost is mostly fixed, avoid tiny tail calls).
        ncalls = -(-nblk // MAX_G_BLK)
        done = 0
        for i in range(ncalls):
            nb = (nblk - done + (ncalls - i) - 1) // (ncalls - i)
            nidx = nb * 128
            s0 = (pos0 + done * 128 - idx_base) // 16
            nc.gpsimd.dma_gather(
                g[:, out_blk0 + done:out_blk0 + done + nb, :], src_view,
                idx_tile[:, s0:s0 + nidx // 16], nidx, nidx, elem,
                queue_num=gq[0])
            gq[0] = (gq[0] + 1) % N_SWDGE_Q
            done += nb

    with tile.TileContext(nc) as tc:
        with tc.tile_pool(name="const", bufs=1) as cpool, \
             tc.tile_pool(name="dram", bufs=1, space="DRAM") as dpool, \
             tc.tile_pool(name="g", bufs=2) as gpool, \
             tc.tile_pool(name="mm", bufs=3) as mpool:
            xT = cpool.tile([128, NPAD], BF16)
            nc.sync.dma_start(xT[:], xT_d)
            w1t = cpool.tile([128, FHID], BF16)
            nc.sync.dma_start(w1t[:], w1t_d)
            w1b = cpool.tile([128, FHID], BF16)
            nc.sync.dma_start(w1b[:], w1b_d)
            w2t = cpool.tile([128, 2 * FOUT], BF16)
            nc.sync.dma_start(w2t[:], w2t_d)
            w2b = cpool.tile([128, 2 * FOUT], BF16)
            nc.sync.dma_start(w2b[:], w2b_d)
            b1c = cpool.tile([128, 2], F32)
            nc.sync.dma_start(b1c[:], b1c_d)
            b2s = cpool.tile([1, FOUT], BF16)
            nc.sync.dma_start(b2s[:], b2_d)
            invb = cpool.tile([128, NPAD], BF16)
            nc.sync.dma_start(invb[:], invb_d)
            invp = cpool.tile([128, T], F32)
            nc.sync.dma_start(invp[:], invp_d)
            i2 = cpool.tile([128, npos // 16], I16)
            nc.sync.dma_start(i2[:], i2_d)
            ones = cpool.tile([1, 128], BF16)
            nc.vector.memset(ones[:], 1.0)

            hTa = cpool.tile([128, NPAD], BF16)
            hTb = cpool.tile([128, NPAD], BF16)
            hwb = dpool.tile([NPAD, FOUT], FP8)
            hwf = dpool.tile([HWROWS, FOUT], FP8, addr_space="Shared")
            mres = []

            # ------------- Layer 1 (h computed transposed) + hW ----------
            with tc.tile_pool(name="paggT", bufs=2, space="PSUM") as paggT_pool, \
                 tc.tile_pool(name="pself", bufs=2, space="PSUM") as pself_pool, \
                 tc.tile_pool(name="phw", bufs=2, space="PSUM") as phw_pool, \
                 tc.tile_pool(name="hn", bufs=2) as hnpool, \
                 tc.tile_pool(name="hwsb", bufs=3) as hwpool:
                for ci, ch in enumerate(meta):
                    g = gpool.tile([128, ch["nblk"], FIN], FP8, tag="g")
                    if ci < RES_CH:
                        mt = cpool.tile([128, ch["nblk"] * 128], FP8,
                                        tag=f"mres{ci}")
                        mres.append(mt)
                    else:
                        mt = mpool.tile([128, ch["nblk"] * 128], FP8, tag="m")
                    nc.sync.dma_start(
                        mt[:], m_d[:, ch["pos0"]:ch["pos0"] + ch["nblk"] * 128])
                    blk0 = ch["pos0"] // 128
                    nc.sync.dma_start(
                        g[:], xe_d[:, blk0:blk0 + ch["nblk"], :])
                    for (t, lo, lnb, hi, hnb) in ch["tiles"]:
                        paggT = paggT_pool.tile([128, 128], F32, tag="paggT")
                        groups = pair_blocks(lo, lnb, hi, hnb)
                        for i, (b, k) in enumerate(groups):
                            nc.tensor.matmul(
                                paggT[:],
                                g[:, b, :] if k == 1 else g[:, b:b + 2, :],
                                m3(mt, b, 1)[:, 0, :] if k == 1 else m3(mt, b, 2),
                                start=(i == 0), stop=(i == len(groups) - 1),
                                perf_mode=None if k == 1 else DR)
                        hn = hnpool.tile([128, 128], BF16, tag="hn")
                        nc.vector.tensor_tensor(
                            hn[:], paggT[:], invb[:, ts(t)], AluOp.mult)
                        for h, hT in ((0, hTa), (1, hTb)):
                            hsl = slice(h * 128, (h + 1) * 128)
                            psT = pself_pool.tile([128, 128], F32, tag="pself")
                            nc.tensor.matmul(psT[:], w1t[:, hsl], xT[:, ts(t)],
                                             start=True, stop=False)
                            nc.tensor.matmul(psT[:], w1b[:, hsl], hn[:],
                                             start=False, stop=True)
                            nc.scalar.activation(hT[:, ts(t)], psT[:],
                                                 ActFn.Relu,
                                                 bias=b1c[:, h:h + 1])
                        ph = phw_pool.tile([128, FOUT], F32, tag="phw")
                        nc.tensor.matmul(ph[:], hTa[:, ts(t)], w2b[:, 0:FOUT],
                                         start=True, stop=False)
                        nc.tensor.matmul(ph[:], hTb[:, ts(t)],
                                         w2b[:, FOUT:2 * FOUT],
                                         start=False, stop=True)
                        hw = hwpool.tile([128, FOUT], FP8, tag="hw")
                        nc.vector.tensor_copy(hw[:], ph[:])
                        nc.sync.dma_start(hwb[ts(t), :], hw[:])

            # Prefetch a few post-resident mask chunks before the collective
            # (DMA serializes around collectives; these land pre-AllGather).
            mpre = {}
            if True:
                for ci in range(RES_CH, min(RES_CH + 3, len(meta))):
                    ch = meta[ci]
                    mt = mpool.tile([128, ch["nblk"] * 128], FP8, tag="m")
                    nc.sync.dma_start(
                        mt[:],
                        m_d[:, ch["pos0"]:ch["pos0"] + ch["nblk"] * 128])
                    mpre[ci] = mt

                nc.gpsimd.collective_compute(
                    "AllGather", AluOp.bypass,
                    replica_groups=[list(range(NCORES))],
                    ins=[hwb.opt()], outs=[hwf.opt()])

                # ---------------- Layer 2 ----------------
                with tc.tile_pool(name="pagg2", bufs=2, space="PSUM") as pagg2_pool, \
                     tc.tile_pool(name="pself2", bufs=2, space="PSUM") as pself2_pool, \
                     tc.tile_pool(name="t1sb", bufs=3) as t1pool, \
                     tc.tile_pool(name="osb", bufs=3) as opool:
                    for ci, ch in enumerate(meta):
                        g = gpool.tile([128, ch["nblk"], FOUT], FP8, tag="g")
                        if ci < RES_CH:
                            mt = mres[ci]
                        elif ci in mpre:
                            mt = mpre.pop(ci)
                        else:
                            mt = mpool.tile([128, ch["nblk"] * 128], FP8,
                                            tag="m")
                            nc.sync.dma_start(
                                mt[:], m_d[:, ch["pos0"]:ch["pos0"] +
                                           ch["nblk"] * 128])
                        if ch["nlow"]:
                            emit_gathers(g, hwf[0:HW_BOUND, :], i2, 0,
                                         ch["pos0"], ch["nlow"], 0, FOUT)
                        if ch["nhigh"]:
                            emit_gathers(g, hwf[HW_BOUND:HWROWS, :], i2, 0,
                                         ch["pos0"] + ch["nlow"] * 128,
                                         ch["nhigh"], ch["nlow"], FOUT)
                        for (t, lo, lnb, hi, hnb) in ch["tiles"]:
                            pagg = pagg2_pool.tile([128, FOUT], F32,
                                                   tag="pagg2")
                            groups = pair_blocks(lo, lnb, hi, hnb)
                            for i, (b, k) in enumerate(groups):
                                nc.tensor.matmul(
                                    pagg[:],
                                    m3(mt, b, 1)[:, 0, :] if k == 1
                                    else m3(mt, b, 2),
                                    g[:, b, :] if k == 1 else g[:, b:b + 2, :],
                                    start=(i == 0),
                                    stop=(i == len(groups) - 1),
                                    perf_mode=None if k == 1 else DR)
                            ps2 = pself2_pool.tile([128, FOUT], F32,
                                                   tag="pself2")
                            nc.tensor.matmul(ps2[:], ones[:1, :], b2s[:1, :],
                                             start=True, stop=False)
                            nc.tensor.matmul(ps2[:], hTa[:, ts(t)],
                                             w2t[:, 0:FOUT],
                                             start=False, stop=False)
                            nc.tensor.matmul(ps2[:], hTb[:, ts(t)],
                                             w2t[:, FOUT:2 * FOUT],
                                             start=False, stop=True)
                            t1 = t1pool.tile([128, FOUT], F32, tag="t1")
                            nc.vector.tensor_scalar(
                                t1[:], pagg[:], invp[:, t:t + 1], None,
                                AluOp.mult)
                            o1 = opool.tile([128, FOUT], F32, tag="o1")
                            nc.vector.tensor_tensor(o1[:], t1[:], ps2[:],
                                                    AluOp.add)
                            o2 = opool.tile([128, FOUT], BF16, tag="o2")
                            nc.scalar.activation(o2[:], o1[:], ActFn.Relu)
                            nc.sync.dma_start(out_d[ts(t), :], o2[:])

    nc.compile()
    return nc


_CACHE = {}


def _run(inputs, trace=False):
    x = np.asarray(inputs["x"], np.float32)
    src = np.asarray(inputs["src"])
    dst = np.asarray(inputs["dst"])
    W1 = np.asarray(inputs["W1"], np.float32)
    b1 = np.asarray(inputs["b1"], np.float32)
    W2 = np.asarray(inputs["W2"], np.float32)
    b2 = np.asarray(inputs["b2"], np.float32)

    deg = np.bincount(dst, minlength=N).astype(np.float64)
    inv_deg = np.where(deg > 0, 1.0 / np.maximum(deg, 1.0), 0.0).astype(np.float32)

    layout, meta, per_core, npos = _plan(src, dst)
    if layout not in _CACHE:
        _CACHE[layout] = _build(layout)
    nc = _CACHE[layout]

    x_bf = x.astype(NP_BF16)
    w1t = np.ascontiguousarray(W1[0:128]).astype(NP_BF16)
    w1b = np.ascontiguousarray(W1[128:256]).astype(NP_BF16)
    w2t = np.ascontiguousarray(
        np.concatenate([W2[0:128], W2[128:256]], axis=1)).astype(NP_BF16)
    w2b = np.ascontiguousarray(
        np.concatenate([W2[256:384], W2[384:512]], axis=1)).astype(NP_BF16)
    b1cr = np.ascontiguousarray(b1.reshape(2, 128).T.astype(np.float32))
    b2r = b2.reshape(1, FOUT).astype(NP_BF16)

    x_f8 = x.astype(NP_FP8)
    in_maps = []
    for c in range(NCORES):
        lows, highs = per_core[c]
        i2w, mpk, gsrc = _fill_core(meta, lows, highs, npos)
        xe = np.ascontiguousarray(
            x_f8[gsrc].reshape(npos // 128, 128, FIN).transpose(1, 0, 2))
        xTc = np.zeros((128, NPAD), NP_BF16)
        xTc[:, :NPC] = x_bf[c * NPC:(c + 1) * NPC].T
        iv = np.zeros(NPAD, np.float32)
        iv[:NPC] = inv_deg[c * NPC:(c + 1) * NPC]
        invb = np.ascontiguousarray(np.tile(iv, (128, 1))).astype(NP_BF16)
        invp = np.ascontiguousarray(iv.reshape(T, 128).T)
        in_maps.append({
            "xe": xe, "xT": xTc,
            "w1t": w1t, "w1b": w1b, "w2t": w2t, "w2b": w2b,
            "b1c": b1cr, "b2": b2r,
            "invb": invb, "invp": invp,
            "i2": i2w, "mpk": mpk,
        })

    res = bass_utils.run_bass_kernel_spmd(
        nc, in_maps, core_ids=list(range(NCORES)), trace=trace)
    out = np.concatenate(
        [res.results[c]["out"][:NPC] for c in range(NCORES)], axis=0)
    return np.ascontiguousarray(out.astype(np.float32)), res


def kernel(**inputs):
    out, _ = _run(inputs, trace=False)
    return out

